# revision 1
# baseline (speedup 1.0000x reference)
"""Discriminative loss kernel for Trainium2 (8 NeuronCores, data-parallel over batch).

Problem: B=8, E=16, H=W=512 (N=262144 pixels), K=32 instance ids (labels 1..32,
0 = background). Each core processes one image:
  pass 1: per-instance counts + center sums (one-hot matmuls on PE),
  pass 2: per-pixel distance-to-own-center -> hinged^2 -> per-instance sums.
Host combines the tiny per-core outputs into the 4 scalar losses.

Canonical pixel chunks: chunk c in [0, 2048) = pixels [c*128, c*128+128).
Device layouts (per core):
  emb_pix [128, 2048, 17] bf16 : [p', c, e] = emb[e, c*128+p'], col 16 = 1.0
  mask_px [128, 16, 128] bf16  : [p', m, P] = label((P*16 + m)*128 + p')
    (i.e. chunk c = P*16 + m)
Outputs: cent [32, 17] f32 = [center sums | counts]; pi [32, 1] f32 = per-inst
sum of hinged^2.
"""
import numpy as np

E = 16
HW = 512
N = HW * HW          # 262144 pixels per image
K = 32
S = 8                # emb DMA slabs
NS = N // S
NC = N // 128        # 2048 chunks
DELTA_VAR, DELTA_DIST = 0.5, 1.5
ALPHA, BETA, GAMMA = 1.0, 1.0, 0.001

_CACHED = {}


def _build():
    from concourse import bass, bacc, mybir, tile, masks

    f32 = mybir.dt.float32
    i32 = mybir.dt.int32

    nc = bacc.Bacc("TRN2", target_bir_lowering=False, debug=False, num_devices=8)
    emb_in = nc.dram_tensor("emb", [E, N], f32, kind="ExternalInput").ap()
    mask_in = nc.dram_tensor("maskD", [128, NC], i32, kind="ExternalInput").ap()
    cent_out = nc.dram_tensor("cent", [K, E + 1], f32, kind="ExternalOutput").ap()
    pi_out = nc.dram_tensor("pi", [128, 4], f32, kind="ExternalOutput").ap()

    with tile.TileContext(nc) as tc:
        _body(nc, tc, bass, mybir, masks, emb_in, mask_in, cent_out, pi_out)
    nc.finalize()
    return nc


def _body(nc, tc, bass, mybir, masks, emb_in, mask_in, cent_out, pi_out):
    f32 = mybir.dt.float32
    bf16 = mybir.dt.bfloat16
    i32 = mybir.dt.int32
    from contextlib import ExitStack

    with ExitStack() as top:
        persist = top.enter_context(tc.tile_pool(name="persist", bufs=1))
        # --- constants ---
        ident = persist.tile([128, 128], bf16)
        masks.make_identity(nc, ident[:])
        iota_k = persist.tile([128, 64, K], bf16)   # [p, chunk-in-window, k] = k+1
        nc.gpsimd.iota(iota_k[:], pattern=[[0, 64], [1, K]], base=1,
                       channel_multiplier=0, allow_small_or_imprecise_dtypes=True)

        # --- residents ---
        emb_pix = persist.tile([128, NC, E + 1], bf16)   # 68KB/partition
        mask_px = persist.tile([128, 16, 128], bf16)
        cext = persist.tile([128, E], bf16)
        cext_bd = persist.tile([128, 4 * E], bf16)       # block-diag centers

        # ---------------- stage 0: mask load + transpose ----------------
        with tc.tile_pool(name="stage0", bufs=2) as st0, \
             tc.tile_pool(name="ps0", bufs=2, space="PSUM") as ps0:
            maskD = st0.tile([128, NC], i32, tag="maskD")
            nc.sync.dma_start(maskD[:], mask_in[:])
            maskb = st0.tile([128, NC], bf16, tag="maskb")
            nc.vector.tensor_copy(maskb[:], maskD[:])
            for g in range(4):  # 4 batches of 4 transposes -> psum [128, 512] bf16
                mps = ps0.tile([128, 512], bf16, tag="mps")
                for b in range(4):
                    m = g * 4 + b
                    nc.tensor.transpose(mps[:, bass.ts(b, 128)],
                                        maskb[:, bass.ts(m, 128)], ident[:])
                nc.vector.tensor_copy(
                    mask_px[:, bass.ts(g, 4), :].rearrange("p a b -> p (a b)"),
                    mps[:])

        # ---------------- pass 1: emb load/transpose + centers ----------------
        # emb slab staging: stg [128=(s,e), 2048] f32; chunk c = s*256 + t
        with tc.tile_pool(name="p1", bufs=4) as p1, \
             tc.tile_pool(name="stgp", bufs=2) as stgp, \
             tc.tile_pool(name="p1psum", bufs=2, space="PSUM") as p1ps, \
             tc.tile_pool(name="centps", bufs=1, space="PSUM") as centps:
            emb_sl = emb_in.rearrange("e (s j) -> e s j", s=S)
            cent = centps.tile([K, E + 1], f32)
            n_mm = [0]

            def cent_mm(lhsT, rhs):
                nc.tensor.matmul(cent[:], lhsT, rhs,
                                 start=(n_mm[0] == 0), stop=(n_mm[0] == NC - 1))
                n_mm[0] += 1

            for w in range(8):  # stg windows of 4096: t in [32w, 32w+32)
                stg = stgp.tile([128, 4096], f32, tag="stg")
                for s_ in range(S):
                    nc.sync.dma_start(stg[16 * s_:16 * s_ + 16, :],
                                      emb_sl[:, s_, bass.ts(w, 4096)])
                stgb = stgp.tile([128, 4096], bf16, tag="stgb")
                nc.scalar.copy(stgb[:], stg[:])
                # 32 transposes; block t' covers chunks {s*256 + 32w + t' : s}
                for h in range(8):
                    eps = p1ps.tile([128, 512], bf16, tag="eps")
                    for b in range(4):
                        tp = 4 * h + b
                        nc.tensor.transpose(eps[:, bass.ts(b, 128)],
                                            stgb[:, bass.ts(tp, 128)], ident[:])
                    # evac: eps[p', b*128 + s*16 + e] -> emb_pix[p', s*256+32w+4h+b, e]
                    nc.scalar.copy(
                        emb_pix[:, :, 0:E].rearrange(
                            "p (s t) e -> p t s e", s=S)[:, 32 * w + 4 * h: 32 * w + 4 * h + 4],
                        eps[:].rearrange("p (b s e) -> p b s e", b=4, s=S))
            nc.vector.memset(emb_pix[:, :, E:E + 1], 1.0)

            # one-hot windows + center matmuls (chunk order c = P*16+m)
            for w in range(32):  # window: c in [64w, 64w+64); P in [4w, 4w+4)
                oh = p1.tile([128, 4, 16, K], bf16, tag="oh")
                mslice = mask_px[:, :, 4 * w:4 * w + 4].rearrange("p m P -> p P m")
                nc.vector.tensor_tensor(
                    out=oh[:],
                    in0=iota_k[:].rearrange("p (a b) k -> p a b k", a=4),
                    in1=mslice.unsqueeze(3).broadcast_to([128, 4, 16, K]),
                    op=mybir.AluOpType.is_equal)
                for a in range(4):
                    for b in range(16):
                        c = 64 * w + 16 * a + b
                        cent_mm(oh[:, a, b, :], emb_pix[:, c, :])

            # derive centers (f32) and bf16 centers_ext replicated x4
            centd = p1.tile([K, E + 1], f32, tag="centd")
            nc.vector.tensor_copy(centd[:], cent[:])
            safec = p1.tile([K, 1], f32, tag="safec")
            nc.vector.tensor_scalar_max(safec[:], centd[:, E:E + 1], 1.0)
            rec = p1.tile([K, 1], f32, tag="rec")
            nc.vector.reciprocal(rec[:], safec[:])
            nc.vector.tensor_scalar(
                out=cext[0:K, :], in0=centd[:, 0:E], scalar1=rec[:],
                scalar2=None, op0=mybir.AluOpType.mult)
            # block-diagonal [128, 64]: cext_bd[(jj,k),(jj',e)] = c[k,e]*[jj==jj']
            nc.vector.memset(cext_bd[:], 0.0)
            for g in range(4):
                nc.sync.dma_start(cext_bd[32 * g:32 * g + K, 16 * g:16 * g + E],
                                  cext[0:K, :])
            nc.sync.dma_start(cent_out[:], centd[:])

        # ---------------- pass 2 ----------------
        import os
        if os.environ.get("K_SKIP_P2"):
            with tc.tile_pool(name="p2s", bufs=1) as p2s:
                pif = p2s.tile([128, 4], f32, tag="pif")
                nc.vector.memset(pif[:], 0.0)
                nc.sync.dma_start(pi_out[:], pif[:])
            return
        P2S = int(os.environ.get("K_P2_STAGE", "9"))
        with tc.tile_pool(name="p2", bufs=3) as p2, \
             tc.tile_pool(name="oh2p", bufs=4) as oh2p, \
             tc.tile_pool(name="ohTp", bufs=3) as ohTp, \
             tc.tile_pool(name="cpxps", bufs=2, space="PSUM") as cpxps, \
             tc.tile_pool(name="ohTps", bufs=2, space="PSUM") as ohTps, \
             tc.tile_pool(name="pips", bufs=1, space="PSUM") as pips:
            pi = pips.tile([128, 4], f32)
            n_pi = [0]
            oh2_tiles = {}
            ohT_tiles = {}
            sq_tile = d_tile = h2_tile = None
            for B4 in range(16):   # h2-batch: chunks [128*B4, 128*B4+128)
                sq_tile = p2.tile([128, 128], f32, tag="sq")
                for Bb in range(4):  # cpx batch: 32 chunks
                    B = 4 * B4 + Bb
                    # (re)generate one-hot window every 2 batches
                    w2 = B // 2
                    if B % 2 == 0:
                        oh2 = oh2p.tile([128, 4, 16, K], bf16, tag="oh2")
                        mslice = mask_px[:, :, 4 * w2:4 * w2 + 4].rearrange(
                            "p m P -> p P m")
                        nc.vector.tensor_tensor(
                            out=oh2[:],
                            in0=iota_k[:].rearrange("p (a b) k -> p a b k", a=4),
                            in1=mslice.unsqueeze(3).broadcast_to([128, 4, 16, K]),
                            op=mybir.AluOpType.is_equal)
                        oh2_tiles[w2] = oh2
                        # transpose to onehotT tile [128, 16, 128]
                        ohT = ohTp.tile([128, 16, 128], bf16, tag="ohT")
                        oh2flat = oh2[:].rearrange("p a b k -> p (a b k)")
                        for g in range(4):
                            ops = ohTps.tile([128, 512], bf16, tag="ops")
                            for b in range(4):
                                blk = 4 * g + b
                                nc.tensor.transpose(ops[:, bass.ts(b, 128)],
                                                    oh2flat[:, bass.ts(blk, 128)],
                                                    ident[:])
                            nc.vector.tensor_copy(
                                ohT[:, bass.ts(g, 4), :].rearrange(
                                    "p a b -> p (a b)"),
                                ops[:])
                        ohT_tiles[w2] = ohT
                    ohT = ohT_tiles[w2]
                    # gather: 8 block-diag MMs -> cpx psum [128, 32, 16] f32
                    cpx = cpxps.tile([128, 32, E], f32, tag="cpx")
                    if P2S >= 2:
                        for bgrel8 in range(8):
                            bgrel = (B % 2) * 8 + bgrel8
                            nc.tensor.matmul(
                                cpx[:, bass.ts(bgrel8, 4), :].rearrange(
                                    "p a b -> p (a b)"),
                                ohT[:, bgrel, :],
                                cext_bd[:],
                                start=True, stop=True)
                    else:
                        nc.vector.memset(cpx[:], 0.0)
                    # diff, square, reduce
                    dif = p2.tile([128, 32, E], bf16, tag="dif")
                    nc.vector.tensor_tensor(
                        out=dif[:], in0=emb_pix[:, bass.ts(B, 32), 0:E],
                        in1=cpx[:], op=mybir.AluOpType.subtract)
                    dsq = p2.tile([128, 32, E], bf16, tag="dsq")
                    nc.vector.tensor_tensor(out=dsq[:], in0=dif[:], in1=dif[:],
                                            op=mybir.AluOpType.mult)
                    nc.vector.tensor_reduce(
                        sq_tile[:, bass.ts(Bb, 32)].unsqueeze(2), dsq[:],
                        axis=mybir.AxisListType.X, op=mybir.AluOpType.add)
                # sqrt -> hinge -> square for 128 chunk-cols
                d_tile = p2.tile([128, 128], bf16, tag="d")
                nc.scalar.sqrt(d_tile[:], sq_tile[:])
                h_tile = p2.tile([128, 128], bf16, tag="h")
                nc.vector.tensor_scalar(
                    out=h_tile[:], in0=d_tile[:], scalar1=DELTA_VAR, scalar2=0.0,
                    op0=mybir.AluOpType.subtract, op1=mybir.AluOpType.max)
                h2_tile = p2.tile([128, 128], bf16, tag="h2")
                nc.scalar.square(h2_tile[:], h_tile[:])
                # per-instance sums for the 2 windows of this batch
                for w3 in (2 * B4, 2 * B4 + 1):
                    oh2 = oh2_tiles.pop(w3)
                    if P2S >= 3:
                        oh2flat = oh2[:].rearrange("p a b k -> p (a b k)")
                        for bgrel in range(16):
                            c0 = 64 * w3 + 4 * bgrel
                            colrel = c0 - 128 * B4
                            nc.tensor.matmul(
                                pi[:], oh2flat[:, bass.ts(bgrel, 128)],
                                h2_tile[:, colrel:colrel + 4],
                                start=(n_pi[0] == 0), stop=(n_pi[0] == 511))
                            n_pi[0] += 1
                    ohT_tiles.pop(w3, None)

            pif = p2.tile([128, 4], f32, tag="pif")
            if P2S >= 3:
                nc.vector.tensor_copy(pif[:], pi[:])
            else:
                nc.vector.memset(pif[:], 0.0)
            nc.sync.dma_start(pi_out[:], pif[:])


def _get_nc():
    if "nc" not in _CACHED:
        _CACHED["nc"] = _build()
    return _CACHED["nc"]


def _host_finish(cents, pis):
    """cents: [8][32,17] f32, pis: [8][32,1] f32 -> loss tuple (float64 math)."""
    B = len(cents)
    lv = np.zeros(B)
    ld = np.zeros(B)
    lr = np.zeros(B)
    valid = np.zeros(B)
    for i in range(B):
        cent = cents[i].astype(np.float64)
        counts = cent[:, E]
        sums = cent[:, :E]
        present = counts > 0.5
        safe_counts = np.maximum(counts, 1.0)
        centers = sums / safe_counts[:, None]
        n_inst = float(present.sum())
        safe_n = max(n_inst, 1.0)
        pi4 = pis[i].astype(np.float64)
        pisum = sum(pi4[32 * jj:32 * jj + K, jj] for jj in range(4))
        per_inst = pisum / safe_counts
        lv[i] = per_inst.sum() / safe_n
        iu = np.arange(K)
        pair = present[:, None] & present[None, :] & (iu[:, None] < iu[None, :])
        dsq = ((centers[:, None, :] - centers[None, :, :]) ** 2).sum(-1)
        dd = np.sqrt(np.where(pair, dsq, 1.0))
        hp = np.maximum(2.0 * DELTA_DIST - dd, 0.0) ** 2 * pair
        n_pairs = n_inst * (n_inst - 1.0) * 0.5
        ld[i] = hp.sum() / max(n_pairs, 1.0)
        cn = np.sqrt(np.where(present, (centers ** 2).sum(-1), 1.0)) * present
        lr[i] = cn.sum() / safe_n
        valid[i] = 1.0 if n_inst > 0 else 0.0
    vb = max(valid.sum(), 1.0)
    L_var = (lv * valid).sum() / vb
    L_dist = (ld * valid).sum() / vb
    L_reg = (lr * valid).sum() / vb
    total = ALPHA * L_var + BETA * L_dist + GAMMA * L_reg
    return (np.float32(total), np.float32(L_var), np.float32(L_dist),
            np.float32(L_reg))


def kernel(embedding, instance_mask):
    from concourse.bass_utils import run_bass_kernel_spmd
    embedding = np.ascontiguousarray(np.asarray(embedding, dtype=np.float32))
    instance_mask = np.ascontiguousarray(np.asarray(instance_mask, dtype=np.int32))
    B = embedding.shape[0]
    assert embedding.shape == (B, E, HW, HW) and instance_mask.shape == (B, HW, HW)
    nc = _get_nc()
    in_maps = []
    for i in range(B):
        in_maps.append({
            "emb": embedding[i].reshape(E, N),
            "maskD": instance_mask[i].reshape(128, NC),
        })
    res = run_bass_kernel_spmd(nc, in_maps, core_ids=list(range(8)))
    cents = [res.results[i]["cent"] for i in range(B)]
    pis = [res.results[i]["pi"] for i in range(B)]
    return _host_finish(cents, pis)


if __name__ == "__main__":
    rng = np.random.default_rng(0)
    emb = rng.standard_normal((8, E, HW, HW)).astype(np.float32)
    mask = rng.integers(0, K + 1, (8, HW, HW)).astype(np.int32)
    out = kernel(emb, mask)
    print("kernel out:", out)



# revision 2
# speedup vs baseline: 13660.6750x; 13660.6750x over previous
"""Discriminative loss kernel for Trainium2 (8 NeuronCores, data-parallel over batch).

Problem: B=8, E=16, H=W=512 (N=262144 pixels), K=32 instance ids (labels 1..32,
0 = background). Each core processes one image:
  pass 1: per-instance counts + center sums (one-hot matmuls on PE),
  pass 2: per-pixel distance-to-own-center -> hinged^2 -> per-instance sums.
Host combines the tiny per-core outputs into the 4 scalar losses.

Canonical pixel chunks: chunk c in [0, 2048) = pixels [c*128, c*128+128).
Device layouts (per core):
  emb_pix [128, 2048, 17] bf16 : [p', c, e] = emb[e, c*128+p'], col 16 = 1.0
  mask_px [128, 16, 128] bf16  : [p', m, P] = label((P*16 + m)*128 + p')
    (i.e. chunk c = P*16 + m)
Outputs: cent [32, 17] f32 = [center sums | counts]; pi [32, 1] f32 = per-inst
sum of hinged^2.
"""
import numpy as np

E = 16
HW = 512
N = HW * HW          # 262144 pixels per image
K = 32
S = 8                # emb DMA slabs
NS = N // S
NC = N // 128        # 2048 chunks
DELTA_VAR, DELTA_DIST = 0.5, 1.5
ALPHA, BETA, GAMMA = 1.0, 1.0, 0.001

_CACHED = {}


def _build():
    from concourse import bass, bacc, mybir, tile, masks

    f32 = mybir.dt.float32
    i32 = mybir.dt.int32

    nc = bacc.Bacc("TRN2", target_bir_lowering=False, debug=False, num_devices=8)
    emb_in = nc.dram_tensor("emb", [E, N], f32, kind="ExternalInput").ap()
    mask_in = nc.dram_tensor("maskD", [128, NC], i32, kind="ExternalInput").ap()
    cent_out = nc.dram_tensor("cent", [K, E + 1], f32, kind="ExternalOutput").ap()
    pi_out = nc.dram_tensor("pi", [128, 4], f32, kind="ExternalOutput").ap()

    with tile.TileContext(nc) as tc:
        _body(nc, tc, bass, mybir, masks, emb_in, mask_in, cent_out, pi_out)
    nc.finalize()
    return nc


def _body(nc, tc, bass, mybir, masks, emb_in, mask_in, cent_out, pi_out):
    f32 = mybir.dt.float32
    bf16 = mybir.dt.bfloat16
    i32 = mybir.dt.int32
    from contextlib import ExitStack

    with ExitStack() as top:
        persist = top.enter_context(tc.tile_pool(name="persist", bufs=1))
        # --- constants ---
        ident = persist.tile([128, 128], bf16)
        masks.make_identity(nc, ident[:])
        iota_k = persist.tile([128, 64, K], bf16)   # [p, chunk-in-window, k] = k+1
        nc.gpsimd.iota(iota_k[:], pattern=[[0, 64], [1, K]], base=1,
                       channel_multiplier=0, allow_small_or_imprecise_dtypes=True)

        # --- residents ---
        emb_pix = persist.tile([128, NC, E + 1], bf16)   # 68KB/partition
        mask_px = persist.tile([128, 16, 128], bf16)
        cext = persist.tile([128, E], bf16)
        cext_bd = persist.tile([128, 4 * E], bf16)       # block-diag centers

        # ---------------- stage 0: mask load + transpose ----------------
        with tc.tile_pool(name="stage0", bufs=2) as st0, \
             tc.tile_pool(name="ps0", bufs=2, space="PSUM") as ps0:
            maskD = st0.tile([128, NC], i32, tag="maskD")
            nc.sync.dma_start(maskD[:], mask_in[:])
            maskb = st0.tile([128, NC], bf16, tag="maskb")
            nc.vector.tensor_copy(maskb[:], maskD[:])
            for g in range(4):  # 4 batches of 4 transposes -> psum [128, 512] bf16
                mps = ps0.tile([128, 512], bf16, tag="mps")
                for b in range(4):
                    m = g * 4 + b
                    nc.tensor.transpose(mps[:, bass.ts(b, 128)],
                                        maskb[:, bass.ts(m, 128)], ident[:])
                nc.vector.tensor_copy(
                    mask_px[:, bass.ts(g, 4), :].rearrange("p a b -> p (a b)"),
                    mps[:])

        # ---------------- pass 1: emb load/transpose + centers ----------------
        # emb slab staging: stg [128=(s,e), 2048] f32; chunk c = s*256 + t
        with tc.tile_pool(name="p1", bufs=4) as p1, \
             tc.tile_pool(name="stgp", bufs=2) as stgp, \
             tc.tile_pool(name="p1psum", bufs=2, space="PSUM") as p1ps, \
             tc.tile_pool(name="centps", bufs=1, space="PSUM") as centps:
            emb_sl = emb_in.rearrange("e (s j) -> e s j", s=S)
            cent = centps.tile([K, E + 1], f32)
            n_mm = [0]

            def cent_mm(lhsT, rhs):
                nc.tensor.matmul(cent[:], lhsT, rhs,
                                 start=(n_mm[0] == 0), stop=(n_mm[0] == NC - 1))
                n_mm[0] += 1

            for w in range(8):  # stg windows of 4096: t in [32w, 32w+32)
                stg = stgp.tile([128, 4096], f32, tag="stg")
                for s_ in range(S):
                    nc.sync.dma_start(stg[16 * s_:16 * s_ + 16, :],
                                      emb_sl[:, s_, bass.ts(w, 4096)])
                stgb = stgp.tile([128, 4096], bf16, tag="stgb")
                nc.scalar.copy(stgb[:], stg[:])
                # 32 transposes; block t' covers chunks {s*256 + 32w + t' : s}
                for h in range(8):
                    eps = p1ps.tile([128, 512], bf16, tag="eps")
                    for b in range(4):
                        tp = 4 * h + b
                        nc.tensor.transpose(eps[:, bass.ts(b, 128)],
                                            stgb[:, bass.ts(tp, 128)], ident[:])
                    # evac: eps[p', b*128 + s*16 + e] -> emb_pix[p', s*256+32w+4h+b, e]
                    nc.scalar.copy(
                        emb_pix[:, :, 0:E].rearrange(
                            "p (s t) e -> p t s e", s=S)[:, 32 * w + 4 * h: 32 * w + 4 * h + 4],
                        eps[:].rearrange("p (b s e) -> p b s e", b=4, s=S))
            nc.vector.memset(emb_pix[:, :, E:E + 1], 1.0)

            # one-hot windows + center matmuls (chunk order c = P*16+m)
            for w in range(32):  # window: c in [64w, 64w+64); P in [4w, 4w+4)
                oh = p1.tile([128, 4, 16, K], bf16, tag="oh")
                mslice = mask_px[:, :, 4 * w:4 * w + 4].rearrange("p m P -> p P m")
                nc.vector.tensor_tensor(
                    out=oh[:],
                    in0=iota_k[:].rearrange("p (a b) k -> p a b k", a=4),
                    in1=mslice.unsqueeze(3).broadcast_to([128, 4, 16, K]),
                    op=mybir.AluOpType.is_equal)
                for a in range(4):
                    for b in range(16):
                        c = 64 * w + 16 * a + b
                        cent_mm(oh[:, a, b, :], emb_pix[:, c, :])

            # derive centers (f32) and bf16 centers_ext replicated x4
            centd = p1.tile([K, E + 1], f32, tag="centd")
            nc.vector.tensor_copy(centd[:], cent[:])
            safec = p1.tile([K, 1], f32, tag="safec")
            nc.vector.tensor_scalar_max(safec[:], centd[:, E:E + 1], 1.0)
            rec = p1.tile([K, 1], f32, tag="rec")
            nc.vector.reciprocal(rec[:], safec[:])
            nc.vector.tensor_scalar(
                out=cext[0:K, :], in0=centd[:, 0:E], scalar1=rec[:],
                scalar2=None, op0=mybir.AluOpType.mult)
            # block-diagonal [128, 64]: cext_bd[(jj,k),(jj',e)] = c[k,e]*[jj==jj']
            nc.vector.memset(cext_bd[:], 0.0)
            for g in range(4):
                nc.sync.dma_start(cext_bd[32 * g:32 * g + K, 16 * g:16 * g + E],
                                  cext[0:K, :])
            nc.sync.dma_start(cent_out[:], centd[:])

        # ---------------- pass 2 ----------------
        import os
        if os.environ.get("K_SKIP_P2"):
            with tc.tile_pool(name="p2s", bufs=1) as p2s:
                pif = p2s.tile([128, 4], f32, tag="pif")
                nc.vector.memset(pif[:], 0.0)
                nc.sync.dma_start(pi_out[:], pif[:])
            return
        P2S = int(os.environ.get("K_P2_STAGE", "9"))
        with tc.tile_pool(name="p2", bufs=3) as p2, \
             tc.tile_pool(name="oh2p", bufs=4) as oh2p, \
             tc.tile_pool(name="ohTp", bufs=3) as ohTp, \
             tc.tile_pool(name="cpxps", bufs=2, space="PSUM") as cpxps, \
             tc.tile_pool(name="ohTps", bufs=2, space="PSUM") as ohTps, \
             tc.tile_pool(name="pips", bufs=1, space="PSUM") as pips:
            pi = pips.tile([128, 4], f32)
            n_pi = [0]
            oh2_tiles = {}
            ohT_tiles = {}
            sq_tile = d_tile = h2_tile = None
            for B4 in range(16):   # h2-batch: chunks [128*B4, 128*B4+128)
                sq_tile = p2.tile([128, 128], f32, tag="sq")
                for Bb in range(4):  # cpx batch: 32 chunks
                    B = 4 * B4 + Bb
                    # (re)generate one-hot window every 2 batches
                    w2 = B // 2
                    if B % 2 == 0:
                        oh2 = oh2p.tile([128, 4, 16, K], bf16, tag="oh2")
                        mslice = mask_px[:, :, 4 * w2:4 * w2 + 4].rearrange(
                            "p m P -> p P m")
                        nc.vector.tensor_tensor(
                            out=oh2[:],
                            in0=iota_k[:].rearrange("p (a b) k -> p a b k", a=4),
                            in1=mslice.unsqueeze(3).broadcast_to([128, 4, 16, K]),
                            op=mybir.AluOpType.is_equal)
                        oh2_tiles[w2] = oh2
                        # transpose to onehotT tile [128, 16, 128]
                        ohT = ohTp.tile([128, 16, 128], bf16, tag="ohT")
                        oh2flat = oh2[:].rearrange("p a b k -> p (a b k)")
                        for g in range(4):
                            ops = ohTps.tile([128, 512], bf16, tag="ops")
                            for b in range(4):
                                blk = 4 * g + b
                                nc.tensor.transpose(ops[:, bass.ts(b, 128)],
                                                    oh2flat[:, bass.ts(blk, 128)],
                                                    ident[:])
                            nc.vector.tensor_copy(
                                ohT[:, bass.ts(g, 4), :].rearrange(
                                    "p a b -> p (a b)"),
                                ops[:])
                        ohT_tiles[w2] = ohT
                    ohT = ohT_tiles[w2]
                    # gather: 8 block-diag MMs -> cpx psum [128, 32, 16] f32
                    cpx = cpxps.tile([128, 32, E], f32, tag="cpx")
                    if P2S >= 2:
                        for bgrel8 in range(8):
                            bgrel = (B % 2) * 8 + bgrel8
                            nc.tensor.matmul(
                                cpx[:, bass.ts(bgrel8, 4), :].rearrange(
                                    "p a b -> p (a b)"),
                                ohT[:, bgrel, :],
                                cext_bd[:],
                                start=True, stop=True)
                    else:
                        nc.vector.memset(cpx[:], 0.0)
                    # diff, square, reduce
                    dif = p2.tile([128, 32, E], bf16, tag="dif")
                    nc.vector.tensor_tensor(
                        out=dif[:], in0=emb_pix[:, bass.ts(B, 32), 0:E],
                        in1=cpx[:], op=mybir.AluOpType.subtract)
                    dsq = p2.tile([128, 32, E], bf16, tag="dsq")
                    nc.vector.tensor_tensor(out=dsq[:], in0=dif[:], in1=dif[:],
                                            op=mybir.AluOpType.mult)
                    nc.vector.tensor_reduce(
                        sq_tile[:, bass.ts(Bb, 32)].unsqueeze(2), dsq[:],
                        axis=mybir.AxisListType.X, op=mybir.AluOpType.add)
                # sqrt -> hinge -> square for 128 chunk-cols
                d_tile = p2.tile([128, 128], bf16, tag="d")
                nc.scalar.sqrt(d_tile[:], sq_tile[:])
                h_tile = p2.tile([128, 128], bf16, tag="h")
                nc.vector.tensor_scalar(
                    out=h_tile[:], in0=d_tile[:], scalar1=DELTA_VAR, scalar2=0.0,
                    op0=mybir.AluOpType.subtract, op1=mybir.AluOpType.max)
                h2_tile = p2.tile([128, 128], bf16, tag="h2")
                nc.scalar.square(h2_tile[:], h_tile[:])
                # per-instance sums for the 2 windows of this batch
                for w3 in (2 * B4, 2 * B4 + 1):
                    oh2 = oh2_tiles.pop(w3)
                    if P2S >= 3:
                        oh2flat = oh2[:].rearrange("p a b k -> p (a b k)")
                        for bgrel in range(16):
                            c0 = 64 * w3 + 4 * bgrel
                            colrel = c0 - 128 * B4
                            nc.tensor.matmul(
                                pi[:], oh2flat[:, bass.ts(bgrel, 128)],
                                h2_tile[:, colrel:colrel + 4],
                                start=(n_pi[0] == 0), stop=(n_pi[0] == 511))
                            n_pi[0] += 1
                    ohT_tiles.pop(w3, None)

            pif = p2.tile([128, 4], f32, tag="pif")
            if P2S >= 3:
                nc.vector.tensor_copy(pif[:], pi[:])
            else:
                nc.vector.memset(pif[:], 0.0)
            nc.sync.dma_start(pi_out[:], pif[:])


def _get_nc():
    if "nc" not in _CACHED:
        _CACHED["nc"] = _build()
    return _CACHED["nc"]


def _host_finish(cents, pis):
    """cents: [8][32,17] f32, pis: [8][32,1] f32 -> loss tuple (float64 math)."""
    B = len(cents)
    lv = np.zeros(B)
    ld = np.zeros(B)
    lr = np.zeros(B)
    valid = np.zeros(B)
    for i in range(B):
        cent = cents[i].astype(np.float64)
        counts = cent[:, E]
        sums = cent[:, :E]
        present = counts > 0.5
        safe_counts = np.maximum(counts, 1.0)
        centers = sums / safe_counts[:, None]
        n_inst = float(present.sum())
        safe_n = max(n_inst, 1.0)
        pi4 = pis[i].astype(np.float64)
        pisum = sum(pi4[32 * jj:32 * jj + K, jj] for jj in range(4))
        per_inst = pisum / safe_counts
        lv[i] = per_inst.sum() / safe_n
        iu = np.arange(K)
        pair = present[:, None] & present[None, :] & (iu[:, None] < iu[None, :])
        dsq = ((centers[:, None, :] - centers[None, :, :]) ** 2).sum(-1)
        dd = np.sqrt(np.where(pair, dsq, 1.0))
        hp = np.maximum(2.0 * DELTA_DIST - dd, 0.0) ** 2 * pair
        n_pairs = n_inst * (n_inst - 1.0) * 0.5
        ld[i] = hp.sum() / max(n_pairs, 1.0)
        cn = np.sqrt(np.where(present, (centers ** 2).sum(-1), 1.0)) * present
        lr[i] = cn.sum() / safe_n
        valid[i] = 1.0 if n_inst > 0 else 0.0
    vb = max(valid.sum(), 1.0)
    L_var = (lv * valid).sum() / vb
    L_dist = (ld * valid).sum() / vb
    L_reg = (lr * valid).sum() / vb
    total = ALPHA * L_var + BETA * L_dist + GAMMA * L_reg
    return (np.float32(total), np.float32(L_var), np.float32(L_dist),
            np.float32(L_reg))


def kernel(embedding, instance_mask):
    import os
    from concourse.bass_utils import run_bass_kernel_spmd
    embedding = np.ascontiguousarray(np.asarray(embedding, dtype=np.float32))
    instance_mask = np.ascontiguousarray(np.asarray(instance_mask, dtype=np.int32))
    B = embedding.shape[0]
    assert embedding.shape == (B, E, HW, HW) and instance_mask.shape == (B, HW, HW)
    nc = _get_nc()
    in_maps = []
    for i in range(B):
        in_maps.append({
            "emb": embedding[i].reshape(E, N),
            "maskD": instance_mask[i].reshape(128, NC),
        })
    trace = bool(os.environ.get("K_TRACE"))
    res = run_bass_kernel_spmd(nc, in_maps, core_ids=list(range(8)), trace=trace)
    _CACHED["last_res"] = res
    cents = [res.results[i]["cent"] for i in range(B)]
    pis = [res.results[i]["pi"] for i in range(B)]
    return _host_finish(cents, pis)


if __name__ == "__main__":
    rng = np.random.default_rng(0)
    emb = rng.standard_normal((8, E, HW, HW)).astype(np.float32)
    mask = rng.integers(0, K + 1, (8, HW, HW)).astype(np.int32)
    out = kernel(emb, mask)
    print("kernel out:", out)

